# revision 18
# baseline (speedup 1.0000x reference)
"""Trainium2 8-core kernel for nn_Attn_user_47863115547245.

reference:
    proj     = id_emb @ attn_W.T + attn_b                  # [seq, hid]
    energies = w1*(user @ proj.T) + w2*(socail @ proj.T)   # [state, seq]
    out      = softmax(energies, axis=-1)

Algebraic restructuring (exact up to float rounding):
  * linearity: energies = (w1*user + w2*socail) @ proj.T
  * reassociation: combined @ (W @ id.T) == (combined @ W) @ id.T, and
    state(2048) < seq(4096) makes (combined @ W) first strictly cheaper.
  * the bias term contributes combined_i . b, constant along the softmax
    axis -> cancels exactly in softmax; dropped.
  * w_big = the larger of |w1|,|w2| is folded into W on the host;
    the ratio w_small/w_big is applied on-device in the combine step.

Sharding: data-parallel over state rows, 256 rows/core x 8 cores.
id_emb (fp16, pre-transposed, s-block-packed) and W (fp16, pre-scaled)
replicated. Softmax is row-local -> zero collectives.

Matmuls run in fp16 with fp32 PSUM accumulation; the softmax
probabilities are written as fp16 and upcast to fp32 on the host
(softmax outputs are in [0,1]; fp16 rounding adds ~2e-4 relative
error on top of the ~2.5e-3 from fp16 matmul inputs).

Perf structure (from NTFF profiles):
  * every host tensor is packed so each DMA is one instruction with
    8-16KB contiguous per-partition runs; DMAs alternate between the
    two HWDGE rings (sync + scalar), which together run at ~HBM rate.
  * mm1 accumulates h-outer into 8 PSUM banks so it streams behind the
    quarter-granular W DMAs, keeping it off the critical path.
  * mm2 iterates seq-chunk-outer / row-tile-inner so the PE work that
    depends on the last id_emb bytes is minimal.
  * softmax is online over NH seq chunks: exp uses the chunk-local max
    immediately; the final per-row rescale exp(Mh-Mtot)/S is folded
    into a dual-scalar normalize, split across DVE and GpSimd.
  * short garbage-matmul bursts bridge PE idle gaps (HAM clock gate).
"""

import numpy as np

STATE, SEQ, HID = 2048, 4096, 1024
NCORES = 8
ROWS = STATE // NCORES        # 256 state rows per core
P = 128                       # partitions
KT = HID // P                 # 8 contraction tiles
MT = ROWS // P                # 2 output row tiles per core
SB = 512                      # seq block (one fp32 PSUM bank)
ST = SEQ // SB                # 8 seq blocks
NH = 4                        # softmax chunks per row tile (online rescale)
SH = ST // NH                 # seq blocks per chunk
WARMUP_MM = 13                # PE warmup matmuls (HAM un-throttle)

_graph_cache: dict = {}


def _build(ratio: float, swap: bool):
    """Build the per-core Bass graph.

    cT = (sT * ratio) + uT  (or roles swapped when |w2|>|w1|), fp16
    tmpT[k,m] = sum_h W'[h,k] * cT[h,m]      (mm1, fp16, h-outer)
    E[m,s]    = sum_k tmpT[k,m] * idT[k,s]   (mm2, fp16, s-chunk-outer)
    out[m,s]  = softmax_s(E)                 (fp16 out, online chunks)
    """
    import concourse.bacc as bacc
    import concourse.mybir as mybir
    import concourse.bass as bass
    from concourse import tile

    f32, f16 = mybir.dt.float32, mybir.dt.float16
    AX = mybir.AxisListType.X
    ALU = mybir.AluOpType
    ACTF = mybir.ActivationFunctionType

    nc = bacc.Bacc()

    uT = nc.declare_dram_parameter("uT", [P, KT, ROWS], f16, isOutput=False)
    sT = nc.declare_dram_parameter("sT", [P, KT, ROWS], f16, isOutput=False)
    Wp = nc.declare_dram_parameter("Wp", [P, KT, HID], f16, isOutput=False)
    idT = nc.declare_dram_parameter("idT", [ST, P, KT, SB], f16, isOutput=False)
    out = nc.declare_dram_parameter("out", [ROWS, SEQ], f16, isOutput=True)

    with tile.TileContext(nc) as tc:
        with (
            tc.tile_pool(name="inp", bufs=1) as inp,
            tc.tile_pool(name="work", bufs=1) as work,
        ):
            # ---- phase-A PSUM pool: warmup + mm1 (5 banks) ----
            with tc.tile_pool(name="psA", bufs=1,
                              space=bass.MemorySpace.PSUM) as psA:
                # PE warmup: garbage matmuls to lift the HAM clock gate
                wgarb = work.tile([P, SB], f16, tag="warmgarb")
                nc.gpsimd.memset(wgarb[:], 0.0)
                psA_big = psA.tile([P, KT, SB], f32)
                for _ in range(WARMUP_MM):
                    nc.tensor.matmul(
                        psA_big[:, 0, :], wgarb[:, :P], wgarb[:],
                        start=True, stop=True)

                # input DMAs, alternating between the two HWDGE rings
                uT_sb = inp.tile([P, KT, ROWS], f16)
                sT_sb = inp.tile([P, KT, ROWS], f16)
                W_sb = inp.tile([P, KT, HID], f16)
                id_sb = inp.tile([P, ST, KT, SB], f16)

                nc.sync.dma_start(uT_sb[:], uT[:])
                nc.scalar.dma_start(sT_sb[:], sT[:])
                for j in range(4):  # W quarters pace the h-outer mm1
                    eng = nc.sync if j % 2 == 0 else nc.scalar
                    eng.dma_start(
                        W_sb[:, 2 * j:2 * j + 2, :], Wp[:, 2 * j:2 * j + 2, :])
                for s in range(ST):
                    eng = nc.sync if s % 2 == 0 else nc.scalar
                    eng.dma_start(id_sb[:, s, :, :], idT[s])

                # combine: cT = (in0 * ratio) + in1, fp16
                cT_sb = work.tile([P, KT, ROWS], f16)
                for k in range(KT):
                    in0 = sT_sb[:, k, :] if not swap else uT_sb[:, k, :]
                    in1 = uT_sb[:, k, :] if not swap else sT_sb[:, k, :]
                    nc.vector.scalar_tensor_tensor(
                        cT_sb[:, k, :], in0, float(ratio), in1,
                        op0=ALU.mult, op1=ALU.add,
                    )

                # mm1 (h-outer): one bank per kb accumulates tmpT over h
                tmpT_sb = work.tile([P, KT, ROWS], f16)
                for h in range(KT):
                    for kb in range(KT):
                        nc.tensor.matmul(
                            psA_big[:, kb, :ROWS],
                            W_sb[:, h, P * kb:P * (kb + 1)],
                            cT_sb[:, h, :],
                            start=(h == 0), stop=(h == KT - 1),
                        )
                # two wide casts (half the per-op overhead of eight)
                nc.vector.tensor_copy(
                    tmpT_sb[:, :KT // 2, :], psA_big[:, :KT // 2, :ROWS])
                nc.vector.tensor_copy(
                    tmpT_sb[:, KT // 2:, :], psA_big[:, KT // 2:, :ROWS])

            # ---- mm2 (seq-chunk-outer, m-inner) + online softmax ----
            # Uneven chunks: a tiny final chunk keeps the post-last-matmul
            # chain (pmax -> exp -> sums -> normalize) short. All engines
            # except PE execute strictly in order, so emission order below
            # IS the tail schedule.
            _psB_cm = tc.tile_pool(name="psB", bufs=8,
                                   space=bass.MemorySpace.PSUM)
            psp = _psB_cm.__enter__()
            CH = [2, 2, 3, 1]             # seq blocks per chunk (sum == ST)
            CO = [0, 2, 4, 7]             # chunk start block
            pun_sb = work.tile([P, MT, SEQ], f16)
            Mh = [work.tile([P, NH], f32, tag=f"Mh{m}", name=f"Mh{m}")
                  for m in range(MT)]
            Sh = [work.tile([P, NH], f32, tag=f"Sh{m}", name=f"Sh{m}")
                  for m in range(MT)]
            rinvs, ehs = {}, {}

            def rescale(m):
                """Per-row rescale factors for row tile m (small ops)."""
                negmtot = work.tile([P, 1], f32, tag=f"negmtot{m}",
                                    name=f"negmtot{m}")
                nc.vector.tensor_reduce(
                    negmtot[:], Mh[m][:], axis=AX, op=ALU.max, negate=True)
                eh = work.tile([P, NH], f32, tag=f"eh{m}", name=f"eh{m}")
                nc.scalar.activation(
                    eh[:], Mh[m][:], ACTF.Exp, bias=negmtot[:], scale=1.0)
                sehs = work.tile([P, NH], f32, tag=f"sehs{m}", name=f"sehs{m}")
                nc.vector.tensor_mul(sehs[:], Sh[m][:], eh[:])
                stot = work.tile([P, 1], f32, tag=f"stot{m}", name=f"stot{m}")
                nc.vector.reduce_sum(stot[:], sehs[:], axis=AX)
                rinv = work.tile([P, 1], f32, tag=f"rinv{m}", name=f"rinv{m}")
                nc.vector.reciprocal(rinv[:], stot[:])
                rinvs[m], ehs[m] = rinv, eh

            def norm_and_out(m, h, veng):
                chunk = slice(SB * CO[h], SB * (CO[h] + CH[h]))
                veng.tensor_scalar(
                    pun_sb[:, m, chunk], pun_sb[:, m, chunk],
                    ehs[m][:, h:h + 1], rinvs[m][:],
                    op0=ALU.mult, op1=ALU.mult,
                )
                deng = nc.sync if (m + h) % 2 == 0 else nc.scalar
                deng.dma_start(
                    out[P * m:P * (m + 1), chunk], pun_sb[:, m, chunk])

            for h in range(NH):
                for m in range(MT):
                    banks = []
                    for si in range(CH[h]):
                        s = CO[h] + si
                        ps2 = psp.tile([P, SB], f32, tag="ps")
                        for k in range(KT):
                            nc.tensor.matmul(
                                ps2[:],
                                tmpT_sb[:, k, P * m:P * (m + 1)],
                                id_sb[:, s, k, :],
                                start=(k == 0), stop=(k == KT - 1),
                            )
                        banks.append(ps2)

                    if CH[h] > 1:
                        pmax = work.tile([P, CH[h]], f32, tag=f"pmax{m}{h}",
                                         name=f"pmax{m}{h}")
                        for si in range(CH[h]):
                            nc.vector.reduce_max(
                                pmax[:, si:si + 1], banks[si][:], axis=AX)
                        nmx = work.tile([P, 1], f32, tag=f"negmax{m}{h}",
                                        name=f"negmax{m}{h}")
                        nc.vector.tensor_reduce(
                            nmx[:], pmax[:], axis=AX, op=ALU.max, negate=True)
                    else:
                        nmx = work.tile([P, 1], f32, tag=f"negmax{m}{h}",
                                        name=f"negmax{m}{h}")
                        nc.vector.reduce_max(
                            nmx[:], banks[0][:], axis=AX, negate=True)
                    nc.vector.tensor_scalar_mul(Mh[m][:, h:h + 1], nmx[:], -1.0)

                    psums = work.tile([P, CH[h]], f32, tag=f"psums{m}{h}",
                                      name=f"psums{m}{h}")
                    for si in range(CH[h]):
                        s = CO[h] + si
                        nc.scalar.activation(
                            pun_sb[:, m, SB * s:SB * (s + 1)],
                            banks[si][:],
                            ACTF.Exp,
                            bias=nmx[:],
                            scale=1.0,
                            accum_out=psums[:, si:si + 1],
                        )
                    nc.vector.reduce_sum(Sh[m][:, h:h + 1], psums[:], axis=AX)

                    if h == NH - 1:
                        rescale(m)
                        if m == 0:
                            # m0's odd-chunk normalizes ride GpSimd so the
                            # DVE FIFO stays clear for m1's critical chain.
                            norm_and_out(0, 1, nc.gpsimd)
                            norm_and_out(0, 3, nc.gpsimd)
                        else:
                            for hh in range(NH):
                                norm_and_out(1, hh, nc.vector)
                            norm_and_out(0, 0, nc.vector)
                            norm_and_out(0, 2, nc.vector)
            _psB_cm.__exit__(None, None, None)

    nc.compile()
    return nc


def _prepare(user_emb, id_emb, socail_uid_emb, attn_W, w1, w2):
    """Host-side sharding + packing. Returns (ratio, swap, in_maps).

    Packed layouts (per-partition contiguous runs -> few big DMA
    descriptors):
      uT/sT: [128, KT, ROWS]   elem [p,k,m] = x[rows0+m, k*128+p]  (fp16)
      Wp:    [128, KT, HID]    elem [p,h,c] = wbig*W[h*128+p, c]   (fp16)
      idT:   [ST, 128, KT, SB] elem [s,p,k,c] = id[s*512+c, k*128+p] (fp16)
    """
    w1 = float(np.asarray(w1))
    w2 = float(np.asarray(w2))
    swap = abs(w2) > abs(w1)
    wbig = w2 if swap else w1
    wsmall = w1 if swap else w2
    ratio = (wsmall / wbig) if wbig != 0.0 else 0.0

    Wp = (np.float32(wbig) * np.asarray(attn_W, np.float32)).astype(np.float16)
    Wp_pack = np.ascontiguousarray(Wp.reshape(KT, P, HID).transpose(1, 0, 2))

    idh = np.asarray(id_emb, np.float32).astype(np.float16)      # [SEQ, HID]
    idT_pack = np.ascontiguousarray(
        idh.reshape(ST, SB, KT, P).transpose(0, 3, 2, 1)         # [s,p,k,c]
    )

    u = np.asarray(user_emb, np.float32).astype(np.float16)
    s_ = np.asarray(socail_uid_emb, np.float32).astype(np.float16)

    in_maps = []
    for i in range(NCORES):
        rows = slice(ROWS * i, ROWS * (i + 1))
        upack = np.ascontiguousarray(
            u[rows].reshape(ROWS, KT, P).transpose(2, 1, 0))
        spack = np.ascontiguousarray(
            s_[rows].reshape(ROWS, KT, P).transpose(2, 1, 0))
        in_maps.append({
            "uT": upack,
            "sT": spack,
            "Wp": Wp_pack,
            "idT": idT_pack,
        })
    return ratio, swap, in_maps


def kernel(user_emb, id_emb, socail_uid_emb, attn_W, attn_b, w1, w2):
    from concourse.bass_utils import run_bass_kernel_spmd

    ratio, swap, in_maps = _prepare(user_emb, id_emb, socail_uid_emb, attn_W, w1, w2)

    key = (round(ratio, 9), swap)
    nc = _graph_cache.get(key)
    if nc is None:
        nc = _build(ratio, swap)
        _graph_cache[key] = nc

    res = run_bass_kernel_spmd(nc, in_maps, core_ids=list(range(NCORES)))
    return np.concatenate(
        [res.results[i]["out"].astype(np.float32) for i in range(NCORES)], axis=0)


# revision 19
# speedup vs baseline: 1.0107x; 1.0107x over previous
"""Trainium2 8-core kernel for nn_Attn_user_47863115547245.

reference:
    proj     = id_emb @ attn_W.T + attn_b                  # [seq, hid]
    energies = w1*(user @ proj.T) + w2*(socail @ proj.T)   # [state, seq]
    out      = softmax(energies, axis=-1)

Algebraic restructuring (exact up to float rounding):
  * linearity: energies = (w1*user + w2*socail) @ proj.T
  * reassociation: combined @ (W @ id.T) == (combined @ W) @ id.T, and
    state(2048) < seq(4096) makes (combined @ W) first strictly cheaper.
  * the bias term contributes combined_i . b, constant along the softmax
    axis -> cancels exactly in softmax; dropped.
  * w_big = the larger of |w1|,|w2| is folded into W on the host;
    the ratio w_small/w_big is applied on-device in the combine step.

Sharding: data-parallel over state rows, 256 rows/core x 8 cores.
id_emb (fp16, pre-transposed, s-block-packed) and W (fp16, pre-scaled)
replicated. Softmax is row-local -> zero collectives.

Matmuls run in fp16 with fp32 PSUM accumulation; the softmax
probabilities are written as fp16 and upcast to fp32 on the host
(softmax outputs are in [0,1]; fp16 rounding adds ~2e-4 relative
error on top of the ~2.5e-3 from fp16 matmul inputs).

Perf structure (from NTFF profiles):
  * every host tensor is packed so each DMA is one instruction with
    8-16KB contiguous per-partition runs; DMAs alternate between the
    two HWDGE rings (sync + scalar), which together run at ~HBM rate.
  * mm1 accumulates h-outer into 8 PSUM banks so it streams behind the
    quarter-granular W DMAs, keeping it off the critical path.
  * mm2 iterates seq-chunk-outer / row-tile-inner so the PE work that
    depends on the last id_emb bytes is minimal.
  * softmax is online over NH seq chunks: exp uses the chunk-local max
    immediately; the final per-row rescale exp(Mh-Mtot)/S is folded
    into a dual-scalar normalize, split across DVE and GpSimd.
  * short garbage-matmul bursts bridge PE idle gaps (HAM clock gate).
"""

import numpy as np

STATE, SEQ, HID = 2048, 4096, 1024
NCORES = 8
ROWS = STATE // NCORES        # 256 state rows per core
P = 128                       # partitions
KT = HID // P                 # 8 contraction tiles
MT = ROWS // P                # 2 output row tiles per core
SB = 512                      # seq block (one fp32 PSUM bank)
ST = SEQ // SB                # 8 seq blocks
NH = 4                        # softmax chunks per row tile (online rescale)
SH = ST // NH                 # seq blocks per chunk
WARMUP_MM = 13                # PE warmup matmuls (HAM un-throttle)

_graph_cache: dict = {}


def _build(ratio: float, swap: bool):
    """Build the per-core Bass graph.

    cT = (sT * ratio) + uT  (or roles swapped when |w2|>|w1|), fp16
    tmpT[k,m] = sum_h W'[h,k] * cT[h,m]      (mm1, fp16, h-outer)
    E[m,s]    = sum_k tmpT[k,m] * idT[k,s]   (mm2, fp16, s-chunk-outer)
    out[m,s]  = softmax_s(E)                 (fp16 out, online chunks)
    """
    import concourse.bacc as bacc
    import concourse.mybir as mybir
    import concourse.bass as bass
    from concourse import tile

    f32, f16 = mybir.dt.float32, mybir.dt.float16
    AX = mybir.AxisListType.X
    ALU = mybir.AluOpType
    ACTF = mybir.ActivationFunctionType

    nc = bacc.Bacc()

    uT = nc.declare_dram_parameter("uT", [P, KT, ROWS], f16, isOutput=False)
    sT = nc.declare_dram_parameter("sT", [P, KT, ROWS], f16, isOutput=False)
    Wp = nc.declare_dram_parameter("Wp", [P, KT, HID], f16, isOutput=False)
    idT = nc.declare_dram_parameter("idT", [ST, P, KT, SB], f16, isOutput=False)
    out = nc.declare_dram_parameter("out", [ROWS, SEQ], f16, isOutput=True)

    with tile.TileContext(nc) as tc:
        with (
            tc.tile_pool(name="inp", bufs=1) as inp,
            tc.tile_pool(name="work", bufs=1) as work,
        ):
            # ---- phase-A PSUM pool: warmup + mm1 (5 banks) ----
            with tc.tile_pool(name="psA", bufs=1,
                              space=bass.MemorySpace.PSUM) as psA:
                # PE warmup: garbage matmuls to lift the HAM clock gate
                wgarb = work.tile([P, SB], f16, tag="warmgarb")
                nc.gpsimd.memset(wgarb[:], 0.0)
                psA_big = psA.tile([P, KT, SB], f32)
                for _ in range(WARMUP_MM):
                    nc.tensor.matmul(
                        psA_big[:, 0, :], wgarb[:, :P], wgarb[:],
                        start=True, stop=True)

                # input DMAs, alternating between the two HWDGE rings
                uT_sb = inp.tile([P, KT, ROWS], f16)
                sT_sb = inp.tile([P, KT, ROWS], f16)
                W_sb = inp.tile([P, KT, HID], f16)
                id_sb = inp.tile([P, ST, KT, SB], f16)

                # Order: first W quarter pair, then u/s halves, then the
                # rest of W. mm1 is h-outer, so step h needs only W[h] and
                # cT[h] -- this ordering lets mm1 start as soon as the first
                # u/s halves and W quarters land (~11us) instead of waiting
                # for everything.
                H2 = KT // 2
                nc.sync.dma_start(W_sb[:, 0:2, :], Wp[:, 0:2, :])
                nc.scalar.dma_start(W_sb[:, 2:4, :], Wp[:, 2:4, :])
                nc.sync.dma_start(uT_sb[:, :H2, :], uT[:, :H2, :])
                nc.scalar.dma_start(sT_sb[:, :H2, :], sT[:, :H2, :])
                nc.sync.dma_start(uT_sb[:, H2:, :], uT[:, H2:, :])
                nc.scalar.dma_start(sT_sb[:, H2:, :], sT[:, H2:, :])
                nc.sync.dma_start(W_sb[:, 4:6, :], Wp[:, 4:6, :])
                nc.scalar.dma_start(W_sb[:, 6:8, :], Wp[:, 6:8, :])
                for s in range(ST):
                    eng = nc.sync if s % 2 == 0 else nc.scalar
                    eng.dma_start(id_sb[:, s, :, :], idT[s])

                # combine: cT = (in0 * ratio) + in1, fp16
                cT_sb = work.tile([P, KT, ROWS], f16)
                for k in range(KT):
                    in0 = sT_sb[:, k, :] if not swap else uT_sb[:, k, :]
                    in1 = uT_sb[:, k, :] if not swap else sT_sb[:, k, :]
                    nc.vector.scalar_tensor_tensor(
                        cT_sb[:, k, :], in0, float(ratio), in1,
                        op0=ALU.mult, op1=ALU.add,
                    )

                # mm1 (h-outer): one bank per kb accumulates tmpT over h
                tmpT_sb = work.tile([P, KT, ROWS], f16)
                for h in range(KT):
                    for kb in range(KT):
                        nc.tensor.matmul(
                            psA_big[:, kb, :ROWS],
                            W_sb[:, h, P * kb:P * (kb + 1)],
                            cT_sb[:, h, :],
                            start=(h == 0), stop=(h == KT - 1),
                        )
                # two wide casts (half the per-op overhead of eight)
                nc.vector.tensor_copy(
                    tmpT_sb[:, :KT // 2, :], psA_big[:, :KT // 2, :ROWS])
                nc.vector.tensor_copy(
                    tmpT_sb[:, KT // 2:, :], psA_big[:, KT // 2:, :ROWS])

            # ---- mm2 (seq-chunk-outer, m-inner) + online softmax ----
            # Uneven chunks: a tiny final chunk keeps the post-last-matmul
            # chain (pmax -> exp -> sums -> normalize) short. All engines
            # except PE execute strictly in order, so emission order below
            # IS the tail schedule.
            _psB_cm = tc.tile_pool(name="psB", bufs=8,
                                   space=bass.MemorySpace.PSUM)
            psp = _psB_cm.__enter__()
            CH = [2, 2, 3, 1]             # seq blocks per chunk (sum == ST)
            CO = [0, 2, 4, 7]             # chunk start block
            pun_sb = work.tile([P, MT, SEQ], f16)
            Mh = [work.tile([P, NH], f32, tag=f"Mh{m}", name=f"Mh{m}")
                  for m in range(MT)]
            Sh = [work.tile([P, NH], f32, tag=f"Sh{m}", name=f"Sh{m}")
                  for m in range(MT)]
            rinvs, ehs = {}, {}

            def rescale(m):
                """Per-row rescale factors for row tile m (small ops)."""
                negmtot = work.tile([P, 1], f32, tag=f"negmtot{m}",
                                    name=f"negmtot{m}")
                nc.vector.tensor_reduce(
                    negmtot[:], Mh[m][:], axis=AX, op=ALU.max, negate=True)
                eh = work.tile([P, NH], f32, tag=f"eh{m}", name=f"eh{m}")
                nc.scalar.activation(
                    eh[:], Mh[m][:], ACTF.Exp, bias=negmtot[:], scale=1.0)
                sehs = work.tile([P, NH], f32, tag=f"sehs{m}", name=f"sehs{m}")
                nc.vector.tensor_mul(sehs[:], Sh[m][:], eh[:])
                stot = work.tile([P, 1], f32, tag=f"stot{m}", name=f"stot{m}")
                nc.vector.reduce_sum(stot[:], sehs[:], axis=AX)
                rinv = work.tile([P, 1], f32, tag=f"rinv{m}", name=f"rinv{m}")
                nc.vector.reciprocal(rinv[:], stot[:])
                rinvs[m], ehs[m] = rinv, eh

            def norm_and_out(m, h, veng):
                chunk = slice(SB * CO[h], SB * (CO[h] + CH[h]))
                veng.tensor_scalar(
                    pun_sb[:, m, chunk], pun_sb[:, m, chunk],
                    ehs[m][:, h:h + 1], rinvs[m][:],
                    op0=ALU.mult, op1=ALU.mult,
                )
                deng = nc.sync if (m + h) % 2 == 0 else nc.scalar
                deng.dma_start(
                    out[P * m:P * (m + 1), chunk], pun_sb[:, m, chunk])

            for h in range(NH):
                for m in range(MT):
                    banks = []
                    for si in range(CH[h]):
                        s = CO[h] + si
                        ps2 = psp.tile([P, SB], f32, tag="ps")
                        for k in range(KT):
                            nc.tensor.matmul(
                                ps2[:],
                                tmpT_sb[:, k, P * m:P * (m + 1)],
                                id_sb[:, s, k, :],
                                start=(k == 0), stop=(k == KT - 1),
                            )
                        banks.append(ps2)

                    if CH[h] > 1:
                        pmax = work.tile([P, CH[h]], f32, tag=f"pmax{m}{h}",
                                         name=f"pmax{m}{h}")
                        for si in range(CH[h]):
                            nc.vector.reduce_max(
                                pmax[:, si:si + 1], banks[si][:], axis=AX)
                        nmx = work.tile([P, 1], f32, tag=f"negmax{m}{h}",
                                        name=f"negmax{m}{h}")
                        nc.vector.tensor_reduce(
                            nmx[:], pmax[:], axis=AX, op=ALU.max, negate=True)
                    else:
                        nmx = work.tile([P, 1], f32, tag=f"negmax{m}{h}",
                                        name=f"negmax{m}{h}")
                        nc.vector.reduce_max(
                            nmx[:], banks[0][:], axis=AX, negate=True)
                    nc.vector.tensor_scalar_mul(Mh[m][:, h:h + 1], nmx[:], -1.0)

                    psums = work.tile([P, CH[h]], f32, tag=f"psums{m}{h}",
                                      name=f"psums{m}{h}")
                    for si in range(CH[h]):
                        s = CO[h] + si
                        nc.scalar.activation(
                            pun_sb[:, m, SB * s:SB * (s + 1)],
                            banks[si][:],
                            ACTF.Exp,
                            bias=nmx[:],
                            scale=1.0,
                            accum_out=psums[:, si:si + 1],
                        )
                    nc.vector.reduce_sum(Sh[m][:, h:h + 1], psums[:], axis=AX)

                    if h == NH - 1:
                        rescale(m)
                        if m == 0:
                            # m0's odd-chunk normalizes ride GpSimd so the
                            # DVE FIFO stays clear for m1's critical chain.
                            norm_and_out(0, 1, nc.gpsimd)
                            norm_and_out(0, 3, nc.gpsimd)
                        else:
                            for hh in range(NH):
                                norm_and_out(1, hh, nc.vector)
                            norm_and_out(0, 0, nc.vector)
                            norm_and_out(0, 2, nc.vector)
            _psB_cm.__exit__(None, None, None)

    nc.compile()
    return nc


def _prepare(user_emb, id_emb, socail_uid_emb, attn_W, w1, w2):
    """Host-side sharding + packing. Returns (ratio, swap, in_maps).

    Packed layouts (per-partition contiguous runs -> few big DMA
    descriptors):
      uT/sT: [128, KT, ROWS]   elem [p,k,m] = x[rows0+m, k*128+p]  (fp16)
      Wp:    [128, KT, HID]    elem [p,h,c] = wbig*W[h*128+p, c]   (fp16)
      idT:   [ST, 128, KT, SB] elem [s,p,k,c] = id[s*512+c, k*128+p] (fp16)
    """
    w1 = float(np.asarray(w1))
    w2 = float(np.asarray(w2))
    swap = abs(w2) > abs(w1)
    wbig = w2 if swap else w1
    wsmall = w1 if swap else w2
    ratio = (wsmall / wbig) if wbig != 0.0 else 0.0

    Wp = (np.float32(wbig) * np.asarray(attn_W, np.float32)).astype(np.float16)
    Wp_pack = np.ascontiguousarray(Wp.reshape(KT, P, HID).transpose(1, 0, 2))

    idh = np.asarray(id_emb, np.float32).astype(np.float16)      # [SEQ, HID]
    idT_pack = np.ascontiguousarray(
        idh.reshape(ST, SB, KT, P).transpose(0, 3, 2, 1)         # [s,p,k,c]
    )

    u = np.asarray(user_emb, np.float32).astype(np.float16)
    s_ = np.asarray(socail_uid_emb, np.float32).astype(np.float16)

    in_maps = []
    for i in range(NCORES):
        rows = slice(ROWS * i, ROWS * (i + 1))
        upack = np.ascontiguousarray(
            u[rows].reshape(ROWS, KT, P).transpose(2, 1, 0))
        spack = np.ascontiguousarray(
            s_[rows].reshape(ROWS, KT, P).transpose(2, 1, 0))
        in_maps.append({
            "uT": upack,
            "sT": spack,
            "Wp": Wp_pack,
            "idT": idT_pack,
        })
    return ratio, swap, in_maps


def kernel(user_emb, id_emb, socail_uid_emb, attn_W, attn_b, w1, w2):
    from concourse.bass_utils import run_bass_kernel_spmd

    ratio, swap, in_maps = _prepare(user_emb, id_emb, socail_uid_emb, attn_W, w1, w2)

    key = (round(ratio, 9), swap)
    nc = _graph_cache.get(key)
    if nc is None:
        nc = _build(ratio, swap)
        _graph_cache[key] = nc

    res = run_bass_kernel_spmd(nc, in_maps, core_ids=list(range(NCORES)))
    return np.concatenate(
        [res.results[i]["out"].astype(np.float32) for i in range(NCORES)], axis=0)
